# revision 1
# baseline (speedup 1.0000x reference)
"""Multi-head attention forward (B=16, S=1024, d=1024, H=16, Dh=64) on 8
Trainium2 NeuronCores, data-parallel over batch (2 batches per core).

Device kernel (per core, bf16 matmuls, fp32 accumulate):
  inputs (host-prepped): XT [d, 2048] = hidden[2c:2c+2].reshape(2048,d).T,
  WqT/WkT/WvT = W.T [in, out], WoT = Wo.T [dv, o]  (all bf16),
  bq, bk [1024] f32, bo2 = bo + Wo @ bv  (bv folded: softmax rows sum to 1).

  QT[dq,t] = WqT.T @ XT (+bq)         KT likewise
  V[t,dv]  = XT.T @ WvT               (stored head-split with a ones column)
  per (batch, head):
    scoresT[s,t] = KT_h.T(sliced) ... = K @ Q.T   (K=dh=64 contraction)
    PT[s,t] = exp(scoresT * 0.125)    (no max-subtract: scores bounded)
    ctxT_aug[dv+1,t] = [V_h | 1].T @ PT   (row dv = softmax denominator)
    ctxT_h = ctxT_aug[:dv] * bcast(1/denominator)
  outT[o,t] = WoT.T @ ctxT (+bo2)  ->  host transposes back.
"""

import os

import numpy as np
import ml_dtypes

import concourse.bass as bass
import concourse.mybir as mybir
import concourse.tile as tile
from concourse import bacc
from concourse.bass_utils import run_bass_kernel_spmd

P = 128
D = 1024
T = 2048  # tokens per core
TB = 1024  # tokens per batch (= S)
H = 16
DH = 64
KD = D // P  # 8 partition-tiles of the d/dv/s dims
NB = T // TB  # batches per core
NCORES = 8

BF16 = mybir.dt.bfloat16
F32 = mybir.dt.float32
EXPF = mybir.ActivationFunctionType.Exp
IDF = mybir.ActivationFunctionType.Identity
MULT = mybir.AluOpType.mult

# test.py hooks
TRACE = False
TRACE_KWARGS = {}
LAST_RESULTS = None

_NC_CACHE = None


def build_nc():
    nc = bacc.Bacc("TRN2", target_bir_lowering=False, debug=False, num_devices=NCORES)

    xt_d = nc.dram_tensor("xt", [D, T], BF16, kind="ExternalInput")
    wqt_d = nc.dram_tensor("wqt", [D, D], BF16, kind="ExternalInput")
    wkt_d = nc.dram_tensor("wkt", [D, D], BF16, kind="ExternalInput")
    wvt_d = nc.dram_tensor("wvt", [D, D], BF16, kind="ExternalInput")
    wot_d = nc.dram_tensor("wot", [D, D], BF16, kind="ExternalInput")
    bq_d = nc.dram_tensor("bq", [D], F32, kind="ExternalInput")
    bk_d = nc.dram_tensor("bk", [D], F32, kind="ExternalInput")
    bo2_d = nc.dram_tensor("bo2", [D], F32, kind="ExternalInput")
    outt_d = nc.dram_tensor("outt", [D, T], F32, kind="ExternalOutput")

    with tile.TileContext(nc) as tc:
        from contextlib import ExitStack

        with ExitStack() as ctx:
            wpool = ctx.enter_context(tc.tile_pool(name="w", bufs=1))
            xpool = ctx.enter_context(tc.tile_pool(name="x", bufs=1))
            qkpool = ctx.enter_context(tc.tile_pool(name="qk", bufs=1))
            vpool = ctx.enter_context(tc.tile_pool(name="v", bufs=1))
            ptpool = ctx.enter_context(tc.tile_pool(name="pt", bufs=2))
            cpool = ctx.enter_context(tc.tile_pool(name="ctx", bufs=1))
            spool = ctx.enter_context(tc.tile_pool(name="small", bufs=1))
            npool = ctx.enter_context(tc.tile_pool(name="norm", bufs=2))
            opool = ctx.enter_context(tc.tile_pool(name="out", bufs=2))
            p1 = ctx.enter_context(tc.tile_pool(name="p1", bufs=2, space="PSUM"))
            p2 = ctx.enter_context(tc.tile_pool(name="p2", bufs=6, space="PSUM"))

            # ---- global loads ----
            xt = [xpool.tile([P, T], BF16, tag=f"xt{k}", name=f"xt{k}") for k in range(KD)]
            wq, wk, wv, wo = (
                [wpool.tile([P, D], BF16, tag=f"w{nm}{k}", name=f"w{nm}{k}") for k in range(KD)]
                for nm in "qkvo"
            )
            for k in range(KD):
                nc.sync.dma_start(wv[k][:], wvt_d[k * P : (k + 1) * P, :])
                nc.sync.dma_start(xt[k][:], xt_d[k * P : (k + 1) * P, :])
            for wt, wd in ((wq, wqt_d), (wk, wkt_d), (wo, wot_d)):
                for k in range(KD):
                    nc.sync.dma_start(wt[k][:], wd[k * P : (k + 1) * P, :])
            bq_sb = spool.tile([P, KD], F32, tag="bq", name="bq_sb")
            bk_sb = spool.tile([P, KD], F32, tag="bk", name="bk_sb")
            bo_sb = spool.tile([P, KD], F32, tag="bo", name="bo_sb")
            for sb, d in ((bq_sb, bq_d), (bk_sb, bk_d), (bo_sb, bo2_d)):
                nc.sync.dma_start(sb[:], d.rearrange("(o p) -> p o", p=P))

            def v_proj_mt(vtiles, bb, mt):
                # V[t_local, dv] head-split + ones column, one 128-token tile
                nc.vector.memset(vtiles[mt][:, :, DH : DH + 1], 1.0)
                for c in range(D // 512):
                    ps = p2.tile([P, 512], F32, tag="p2", name="p2v")
                    for k in range(KD):
                        nc.tensor.matmul(
                            ps[:],
                            xt[k][:, (bb * KD + mt) * P : (bb * KD + mt + 1) * P],
                            wv[k][:, c * 512 : (c + 1) * 512],
                            start=(k == 0),
                            stop=(k == KD - 1),
                        )
                    nc.vector.tensor_copy(
                        vtiles[mt][:, c * 8 : (c + 1) * 8, 0:DH],
                        ps.rearrange("p (h d) -> p h d", d=DH),
                    )

            prev_out = None
            for b in range(NB):
                v = [
                    vpool.tile([P, H, DH + 1], BF16, tag=f"v{mt}", name=f"v{mt}")
                    for mt in range(KD)
                ]
                for mt in range(KD):
                    v_proj_mt(v, b, mt)

                # ---- attention (Q/K projection fused per head-pair) ----
                ctxt = [cpool.tile([P, TB], BF16, tag=f"ctxt{m}", name=f"ctxt{m}") for m in range(KD)]
                pending = None  # deferred normalization of previous head

                def normalize(pend):
                    h, pvps = pend
                    pti, row0 = h // 2, (h % 2) * DH
                    for c in range(TB // 512):
                        rs = npool.tile([1, 512], F32, tag="rs", name="rs", bufs=1)
                        nc.vector.tensor_copy(rs[:], pvps[c][DH : DH + 1, :])
                        rr = npool.tile([1, 512], F32, tag="rr", name="rr", bufs=1)
                        nc.vector.reciprocal_approx_fast(rr[:], rs[:])
                        rb = npool.tile([DH, 512], F32, tag="rb", name="rb")
                        nc.gpsimd.partition_broadcast(rb[:], rr[:])
                        if row0 == 0:
                            nc.vector.tensor_tensor(
                                ctxt[pti][0:DH, c * 512 : (c + 1) * 512],
                                pvps[c][0:DH, :],
                                rb[:],
                                MULT,
                            )
                        else:
                            ch = npool.tile([DH, 512], BF16, tag="ctxh", name="ctxh")
                            nc.vector.tensor_tensor(
                                ch[:], pvps[c][0:DH, :], rb[:], MULT
                            )
                            nc.sync.dma_start(
                                ctxt[pti][row0 : row0 + DH, c * 512 : (c + 1) * 512],
                                ch[:],
                            )

                class ProjStream:
                    """Incremental 128-row projection: emit 2 k-steps per
                    call (8 MMs total per step over both 512-chunks)."""

                    def __init__(self, j, wt, bias_sb, tag):
                        self.j, self.wt, self.bias_sb = j, wt, bias_sb
                        self.dest = qkpool.tile(
                            [P, TB], BF16, tag=tag, name=tag, bufs=2
                        )
                        self.ps = [
                            p2.tile([P, 512], F32, tag="p2", name="p2proj")
                            for _ in range(TB // 512)
                        ]
                        self.k = 0

                    def step(self, nk=2):
                        j = self.j
                        for k in range(self.k, min(self.k + nk, KD)):
                            for c in range(TB // 512):
                                nc.tensor.matmul(
                                    self.ps[c][:],
                                    self.wt[k][:, j * P : (j + 1) * P],
                                    xt[k][
                                        :, b * TB + c * 512 : b * TB + (c + 1) * 512
                                    ],
                                    start=(k == 0),
                                    stop=(k == KD - 1),
                                )
                        self.k = min(self.k + nk, KD)
                        if self.k == KD and self.ps is not None:
                            for c in range(TB // 512):
                                nc.vector.tensor_scalar_add(
                                    self.dest[:, c * 512 : (c + 1) * 512],
                                    self.ps[c][:],
                                    self.bias_sb[:, j : j + 1],
                                )
                            self.ps = None
                        return self.dest

                def qk_proj(j):
                    q = ProjStream(j, wq, bq_sb, "qtj")
                    q.step(KD)
                    k = ProjStream(j, wk, bk_sb, "ktj")
                    k.step(KD)
                    return q.dest, k.dest

                qk_next = qk_proj(0)
                pending2 = None  # (h, pvps) pairs awaiting normalization
                consume_out = prev_out
                prev_out = None

                for j in range(H // 2):
                    h0, h1 = 2 * j, 2 * j + 1
                    qtj, ktj = qk_next
                    pts = {h0: [], h1: []}

                    def emit_scores(st, pts=pts, qtj=qtj, ktj=ktj, h0=h0, h1=h1):
                        pt_a = ptpool.tile([P, TB], BF16, tag=f"pt{st}", name=f"pt{st}a")
                        pt_b = ptpool.tile([P, TB], BF16, tag=f"pt{st}", name=f"pt{st}b")
                        for c in range(TB // 512):
                            for row0, pt_t in ((0, pt_a), (DH, pt_b)):
                                sps = p1.tile([P, 512], F32, tag="p1", name="p1s")
                                nc.tensor.matmul(
                                    sps[:],
                                    ktj[row0 : row0 + DH, st * P : (st + 1) * P],
                                    qtj[row0 : row0 + DH, c * 512 : (c + 1) * 512],
                                    start=True,
                                    stop=True,
                                )
                                nc.scalar.activation(
                                    pt_t[:, c * 512 : (c + 1) * 512],
                                    sps[:],
                                    EXPF,
                                    scale=0.125,
                                )
                        pts[h0].append(pt_a)
                        pts[h1].append(pt_b)

                    emit_scores(0)
                    if pending2 is not None:
                        for pend in pending2:
                            normalize(pend)
                        pending2 = None
                    pvps = {
                        h: [
                            p2.tile([P, 512], F32, tag="p2", name="p2t")
                            for _ in range(TB // 512)
                        ]
                        for h in (h0, h1)
                    }

                    def pv_step(h, st, pts=pts, pvps=pvps):
                        for c in range(TB // 512):
                            nc.tensor.matmul(
                                pvps[h][c][0 : DH + 1, :],
                                v[st][:, h, :],
                                pts[h][st][:, c * 512 : (c + 1) * 512],
                                start=(st == 0),
                                stop=(st == KD - 1),
                            )

                    if j + 1 < H // 2:
                        nextq = ProjStream(j + 1, wq, bq_sb, "qtj")
                        nextk = ProjStream(j + 1, wk, bk_sb, "ktj")
                    for st in range(1, KD):
                        emit_scores(st)
                        if j + 1 < H // 2:
                            if st < 4:
                                nextq.step(2)
                            else:
                                nextk.step(2)
                        if j == 0 and consume_out is not None:
                            og, op = consume_out
                            for _ in range(2):
                                if op:
                                    og(*op.pop(0))
                        pv_step(h0, st - 1)
                        pv_step(h1, st - 1)
                    pv_step(h0, KD - 1)
                    pv_step(h1, KD - 1)
                    if j == 0 and consume_out is not None:
                        og, op = consume_out
                        while op:
                            og(*op.pop(0))
                        consume_out = None
                    if j + 1 < H // 2:
                        nextq.step(2)
                        nextk.step(2)
                        qk_next = (nextq.dest, nextk.dest)
                    pending2 = ((h0, pvps[h0]), (h1, pvps[h1]))

                for pend in pending2:
                    normalize(pend)

                # ---- output projection: outT[o, t_local], deferred groups ----
                def out_group(mo, c, ctxt=ctxt, bb=b):
                    ps = p2.tile([P, 512], F32, tag="p2", name="p2o")
                    for k in range(KD):
                        nc.tensor.matmul(
                            ps[:],
                            wo[k][:, mo * P : (mo + 1) * P],
                            ctxt[k][:, c * 512 : (c + 1) * 512],
                            start=(k == 0),
                            stop=(k == KD - 1),
                        )
                    osb = opool.tile([P, 512], F32, tag="osb", name="osb")
                    nc.vector.tensor_scalar_add(osb[:], ps[:], bo_sb[:, mo : mo + 1])
                    nc.sync.dma_start(
                        outt_d[
                            mo * P : (mo + 1) * P,
                            bb * TB + c * 512 : bb * TB + (c + 1) * 512,
                        ],
                        osb[:],
                    )

                out_pending = [
                    (mo, c) for mo in range(KD) for c in range(TB // 512)
                ]
                if b + 1 == NB:
                    for mo, c in out_pending:
                        out_group(mo, c)
                else:
                    prev_out = (out_group, out_pending)

    nc.compile()
    return nc


def _get_nc():
    global _NC_CACHE
    if _NC_CACHE is None:
        _NC_CACHE = build_nc()
    return _NC_CACHE


def kernel(hidden_states, Wq, bq, Wk, bk, Wv, bv, Wo, bo):
    global LAST_RESULTS
    bf = ml_dtypes.bfloat16
    hs = np.asarray(hidden_states, np.float32)
    Wq = np.asarray(Wq, np.float32)
    Wk = np.asarray(Wk, np.float32)
    Wv = np.asarray(Wv, np.float32)
    Wo = np.asarray(Wo, np.float32)
    bq = np.asarray(bq, np.float32)
    bk = np.asarray(bk, np.float32)
    bv = np.asarray(bv, np.float32)
    bo = np.asarray(bo, np.float32)

    wqt = np.ascontiguousarray(Wq.T).astype(bf)
    wkt = np.ascontiguousarray(Wk.T).astype(bf)
    wvt = np.ascontiguousarray(Wv.T).astype(bf)
    wot = np.ascontiguousarray(Wo.T).astype(bf)
    bo2 = (bo + Wo @ bv).astype(np.float32)

    bpc = hs.shape[0] // NCORES  # batches per core
    in_maps = []
    for c in range(NCORES):
        xc = hs[c * bpc : (c + 1) * bpc].reshape(bpc * TB, D)
        in_maps.append(
            {
                "xt": np.ascontiguousarray(xc.T).astype(bf),
                "wqt": wqt,
                "wkt": wkt,
                "wvt": wvt,
                "wot": wot,
                "bq": bq,
                "bk": bk,
                "bo2": bo2,
            }
        )

    nc = _get_nc()
    res = run_bass_kernel_spmd(
        nc,
        in_maps,
        core_ids=list(range(NCORES)),
        trace=TRACE,
        **TRACE_KWARGS,
    )
    LAST_RESULTS = res

    out = np.empty((hs.shape[0], TB, D), np.float32)
    for c in range(NCORES):
        ot = res.results[c]["outt"]  # [D, T]
        for b in range(bpc):
            out[c * bpc + b] = ot[:, b * TB : (b + 1) * TB].T
    return out



# revision 12
# speedup vs baseline: 1.0667x; 1.0667x over previous
"""Multi-head attention forward (B=16, S=1024, d=1024, H=16, Dh=64) on 8
Trainium2 NeuronCores, data-parallel over batch (2 batches per core).

Device kernel (per core, bf16 matmuls, fp32 accumulate):
  inputs (host-prepped): XT [d, 2048] = hidden[2c:2c+2].reshape(2048,d).T,
  WqT/WkT/WvT = W.T [in, out], WoT = Wo.T [dv, o]  (all bf16),
  bq, bk [1024] f32, bo2 = bo + Wo @ bv  (bv folded: softmax rows sum to 1).

Key structure (vs earlier version): query chunks (c) are OUTER, head pairs
(j) inner.  Scores for a head pair go into ONE [128,1024] PSUM tile
(h0 -> bank A cols 0:512, h1 -> bank B) so the two 64x128 row-tiled matmuls
become ready together and execute CONCURRENTLY on the PE (row tiles 0/64),
and a single [128,1024] exp drains both.  pvps needs only 2 banks per (c,j)
so PSUM = scores ring 4 + pv 2 + proj 2 = 8 banks.  Projections / V-proj /
out-proj are emitted as 8-matmul fill units drained between score blocks.
"""

import numpy as np
import ml_dtypes

import concourse.bass as bass
import concourse.mybir as mybir
import concourse.tile as tile
from concourse import bacc
from concourse.bass_utils import run_bass_kernel_spmd

P = 128
D = 1024
T = 2048  # tokens per core
TB = 1024  # tokens per batch (= S)
H = 16
DH = 64
KD = D // P  # 8 partition-tiles of the d/dv/s dims
NB = T // TB  # batches per core
CW = 512  # query-chunk width (one PSUM bank of fp32)
NCH = TB // CW  # 2 query chunks per batch
NCORES = 8

BF16 = mybir.dt.bfloat16
F32 = mybir.dt.float32
EXPF = mybir.ActivationFunctionType.Exp
MULT = mybir.AluOpType.mult

# test.py hooks
TRACE = False
TRACE_KWARGS = {}
LAST_RESULTS = None

_NC_CACHE = None


def build_nc():
    from collections import deque
    from contextlib import ExitStack

    nc = bacc.Bacc("TRN2", target_bir_lowering=False, debug=False, num_devices=NCORES)

    xt_d = nc.dram_tensor("xt", [D, T], BF16, kind="ExternalInput")
    wqt_d = nc.dram_tensor("wqt", [D, D], BF16, kind="ExternalInput")
    wkt_d = nc.dram_tensor("wkt", [D, D], BF16, kind="ExternalInput")
    wvt_d = nc.dram_tensor("wvt", [D, D], BF16, kind="ExternalInput")
    wot_d = nc.dram_tensor("wot", [D, D], BF16, kind="ExternalInput")
    bq_d = nc.dram_tensor("bq", [D], F32, kind="ExternalInput")
    bk_d = nc.dram_tensor("bk", [D], F32, kind="ExternalInput")
    bo2_d = nc.dram_tensor("bo2", [D], F32, kind="ExternalInput")
    outt_d = nc.dram_tensor("outt", [D, T], F32, kind="ExternalOutput")

    with tile.TileContext(nc) as tc:
        with ExitStack() as ctx:
            # PSUM first so the 2-bank scores tiles land bank-aligned.
            scp = ctx.enter_context(tc.tile_pool(name="sc", bufs=2, space="PSUM"))
            pvp = ctx.enter_context(tc.tile_pool(name="pv", bufs=1, space="PSUM"))
            prp = ctx.enter_context(tc.tile_pool(name="pr", bufs=2, space="PSUM"))
            wpool = ctx.enter_context(tc.tile_pool(name="w", bufs=1))
            xpool = ctx.enter_context(tc.tile_pool(name="x", bufs=1))
            qkpool = ctx.enter_context(tc.tile_pool(name="qk", bufs=1))
            vpool = ctx.enter_context(tc.tile_pool(name="v", bufs=2))
            ptpool = ctx.enter_context(tc.tile_pool(name="pt", bufs=4))
            cpool = ctx.enter_context(tc.tile_pool(name="ctx", bufs=2))
            npool = ctx.enter_context(tc.tile_pool(name="norm", bufs=2))
            opool = ctx.enter_context(tc.tile_pool(name="out", bufs=2))
            spool = ctx.enter_context(tc.tile_pool(name="small", bufs=1))

            # ---- global tiles + DMA loads (interleaved for fast rampup) ----
            xt = [xpool.tile([P, T], BF16, tag=f"xt{k}", name=f"xt{k}") for k in range(KD)]
            wq, wk, wv, wo = (
                [wpool.tile([P, D], BF16, tag=f"w{nm}{k}", name=f"w{nm}{k}") for k in range(KD)]
                for nm in "qkvo"
            )
            for k in range(KD):
                nc.sync.dma_start(xt[k][:], xt_d[k * P : (k + 1) * P, :])
                nc.sync.dma_start(wq[k][:], wqt_d[k * P : (k + 1) * P, :])
                nc.sync.dma_start(wk[k][:], wkt_d[k * P : (k + 1) * P, :])
            bq_sb = spool.tile([P, KD], F32, tag="bq", name="bq_sb")
            bk_sb = spool.tile([P, KD], F32, tag="bk", name="bk_sb")
            bo_sb = spool.tile([P, KD], F32, tag="bo", name="bo_sb")
            for sb, dr in ((bq_sb, bq_d), (bk_sb, bk_d)):
                nc.sync.dma_start(sb[:], dr.rearrange("(o p) -> p o", p=P))
            for k in range(KD):
                nc.sync.dma_start(wv[k][:], wvt_d[k * P : (k + 1) * P, :])
            nc.sync.dma_start(bo_sb[:], bo2_d.rearrange("(o p) -> p o", p=P))
            for k in range(KD):
                nc.sync.dma_start(wo[k][:], wot_d[k * P : (k + 1) * P, :])

            # ---- persistent per-batch state ----
            vt = {}  # (b, mt) -> v tile [P, H, DH+1]
            kt = {}  # (b, j) -> K^T tile [P, TB]
            qt = {}  # (b, j, c) -> Q^T chunk tile [P, CW]
            ctxts = {}  # (b, j) -> ctx^T tile [P, TB]

            fill = deque()  # (key, closure)
            done = set()

            def push(key, closure):
                fill.append((key, closure))

            def drain(n):
                for _ in range(min(n, len(fill))):
                    k, f = fill.popleft()
                    f()
                    done.add(k)

            def need(key):
                # force-drain (in FIFO order) until `key` has been emitted;
                # guarantees emission-order dependencies for dict tiles.
                while key not in done:
                    assert fill, f"need({key}) but fill queue empty"
                    k, f = fill.popleft()
                    f()
                    done.add(k)

            # ---- fill units (each ~8 matmuls + epilogue) ----
            def v_unit(b, mt, ch):
                def emit():
                    if (b, mt) not in vt:
                        vt[(b, mt)] = vpool.tile(
                            [P, H, DH + 1], BF16, tag=f"v{mt}", name=f"v{mt}", bufs=2
                        )
                        nc.vector.memset(vt[(b, mt)][:, :, DH : DH + 1], 1.0)
                    ps = prp.tile([P, CW], F32, tag="pr", name="prv")
                    for k in range(KD):
                        nc.tensor.matmul(
                            ps[:],
                            xt[k][:, (b * KD + mt) * P : (b * KD + mt + 1) * P],
                            wv[k][:, ch * CW : (ch + 1) * CW],
                            start=(k == 0),
                            stop=(k == KD - 1),
                        )
                    nc.vector.tensor_copy(
                        vt[(b, mt)][:, ch * 8 : (ch + 1) * 8, 0:DH],
                        ps.rearrange("p (h d) -> p h d", d=DH),
                    )

                return emit

            def k_unit(b, j, ch):
                def emit():
                    if (b, j) not in kt:
                        kt[(b, j)] = qkpool.tile(
                            [P, TB], BF16, tag=f"k{j}", name=f"kt{j}", bufs=1
                        )
                    ps = prp.tile([P, CW], F32, tag="pr", name="prk")
                    for k in range(KD):
                        nc.tensor.matmul(
                            ps[:],
                            wk[k][:, j * P : (j + 1) * P],
                            xt[k][:, b * TB + ch * CW : b * TB + (ch + 1) * CW],
                            start=(k == 0),
                            stop=(k == KD - 1),
                        )
                    nc.vector.tensor_scalar_add(
                        kt[(b, j)][:, ch * CW : (ch + 1) * CW], ps[:], bk_sb[:, j : j + 1]
                    )

                return emit

            def q_unit(b, j, c):
                def emit():
                    qt[(b, j, c)] = qkpool.tile(
                        [P, CW], BF16, tag=f"q{j}", name=f"qt{j}", bufs=1
                    )
                    ps = prp.tile([P, CW], F32, tag="pr", name="prq")
                    for k in range(KD):
                        nc.tensor.matmul(
                            ps[:],
                            wq[k][:, j * P : (j + 1) * P],
                            xt[k][:, b * TB + c * CW : b * TB + (c + 1) * CW],
                            start=(k == 0),
                            stop=(k == KD - 1),
                        )
                    nc.vector.tensor_scalar_add(
                        qt[(b, j, c)][:], ps[:], bq_sb[:, j : j + 1]
                    )

                return emit

            def out_unit(b, c, mo):
                def emit():
                    ps = prp.tile([P, CW], F32, tag="pr", name="pro")
                    for k in range(KD):
                        nc.tensor.matmul(
                            ps[:],
                            wo[k][:, mo * P : (mo + 1) * P],
                            ctxts[(b, k)][:, c * CW : (c + 1) * CW],
                            start=(k == 0),
                            stop=(k == KD - 1),
                        )
                    osb = opool.tile([P, CW], F32, tag="osb", name="osb")
                    nc.vector.tensor_scalar_add(osb[:], ps[:], bo_sb[:, mo : mo + 1])
                    nc.sync.dma_start(
                        outt_d[
                            mo * P : (mo + 1) * P,
                            b * TB + c * CW : b * TB + (c + 1) * CW,
                        ],
                        osb[:],
                    )

                return emit

            # ---- attention inner loop ----
            def normalize(b, c, j, pva, pvb):
                if (b, j) not in ctxts:
                    ctxts[(b, j)] = cpool.tile(
                        [P, TB], BF16, tag=f"ctxt{j}", name=f"ctxt{j}", bufs=2
                    )
                ctile = ctxts[(b, j)]
                for h, pv_t in ((0, pva), (1, pvb)):
                    rs = npool.tile([1, CW], F32, tag="rs", name="rs", bufs=1)
                    nc.vector.tensor_copy(rs[:], pv_t[DH : DH + 1, :])
                    rr = npool.tile([1, CW], F32, tag="rr", name="rr", bufs=1)
                    nc.vector.reciprocal_approx_fast(rr[:], rs[:])
                    rb = npool.tile([DH, CW], F32, tag="rb", name="rb", bufs=2)
                    nc.gpsimd.partition_broadcast(rb[:], rr[:])
                    if h == 0:
                        nc.vector.tensor_tensor(
                            ctile[0:DH, c * CW : (c + 1) * CW],
                            pv_t[0:DH, :],
                            rb[:],
                            MULT,
                        )
                    else:
                        ch = npool.tile([DH, CW], BF16, tag="ch", name="ch", bufs=2)
                        nc.vector.tensor_tensor(ch[:], pv_t[0:DH, :], rb[:], MULT)
                        nc.sync.dma_start(
                            ctile[DH:P, c * CW : (c + 1) * CW], ch[:]
                        )

            def attention_cj(b, c, j):
                need(("q", b, j, c))
                need(("k", b, j, 0))
                need(("k", b, j, 1))
                pva = pvp.tile([P, CW], F32, tag="pva", name="pva", bufs=1)
                pvb = pvp.tile([P, CW], F32, tag="pvb", name="pvb", bufs=1)
                ktj = kt[(b, j)]
                qjc = qt[(b, j, c)]
                pts = []
                for blk in range(KD // 2):
                    two = []
                    for st in (2 * blk, 2 * blk + 1):
                        sc = scp.tile([P, 2 * CW], F32, tag="sc", name="sc", bufs=2)
                        for h in range(2):
                            r0 = h * DH
                            nc.tensor.matmul(
                                sc[:, h * CW : (h + 1) * CW],
                                ktj[r0 : r0 + DH, st * P : (st + 1) * P],
                                qjc[r0 : r0 + DH, :],
                                start=True,
                                stop=True,
                            )
                        pt = ptpool.tile([P, 2 * CW], BF16, tag="pt", name="pt", bufs=3)
                        nc.scalar.activation(pt[:], sc[:], EXPF, scale=0.125)
                        pts.append(pt)
                        two.append(st)
                    drain(2)
                    for st in two:
                        need(("v", b, st, j // 4))
                        for h, pv_t in ((0, pva), (1, pvb)):
                            nc.tensor.matmul(
                                pv_t[0 : DH + 1, :],
                                vt[(b, st)][:, 2 * j + h, :],
                                pts[st][:, h * CW : (h + 1) * CW],
                                start=(st == 0),
                                stop=(st == KD - 1),
                            )
                normalize(b, c, j, pva, pvb)

            # ---- head: QK proj of (b0, j0) emitted directly; V + j1 proj
            # queued so early score pairs preempt them by priority ----
            for key, u in (
                (("q", 0, 0, 0), q_unit(0, 0, 0)),
                (("k", 0, 0, 0), k_unit(0, 0, 0)),
                (("k", 0, 0, 1), k_unit(0, 0, 1)),
            ):
                u()
                done.add(key)
            push(("v", 0, 0, 0), v_unit(0, 0, 0))
            push(("v", 0, 1, 0), v_unit(0, 1, 0))
            push(("q", 0, 1, 0), q_unit(0, 1, 0))
            push(("k", 0, 1, 0), k_unit(0, 1, 0))
            push(("k", 0, 1, 1), k_unit(0, 1, 1))
            for mt in range(2, KD):
                push(("v", 0, mt, 0), v_unit(0, mt, 0))
            for mt in range(KD):
                push(("v", 0, mt, 1), v_unit(0, mt, 1))

            # ---- main loops ----
            for b in range(NB):
                for c in range(NCH):
                    for j in range(KD):
                        # schedule fill production
                        if c == 0:
                            if j < KD - 1:
                                if not (b == 0 and j == 0):  # j1 pre-queued in head
                                    push(("q", b, j + 1, 0), q_unit(b, j + 1, 0))
                                    push(("k", b, j + 1, 0), k_unit(b, j + 1, 0))
                                    push(("k", b, j + 1, 1), k_unit(b, j + 1, 1))
                            else:
                                push(("q", b, 0, 1), q_unit(b, 0, 1))
                        else:
                            if j < KD - 1:
                                push(("q", b, j + 1, 1), q_unit(b, j + 1, 1))
                            if j == 0 and b + 1 < NB:
                                # next batch V-proj + this batch's c0 out-proj
                                for mt in range(KD):
                                    push(("v", b + 1, mt, 0), v_unit(b + 1, mt, 0))
                                    if mt % 2 == 0:
                                        push(("o", b, 0, mt // 2), out_unit(b, 0, mt // 2))
                                for mt in range(KD):
                                    push(("v", b + 1, mt, 1), v_unit(b + 1, mt, 1))
                                    if mt % 2 == 1:
                                        push(("o", b, 0, mt // 2 + 4), out_unit(b, 0, mt // 2 + 4))
                            if j == 0 and b + 1 == NB:
                                for mo in range(KD):
                                    push(("o", b, 0, mo), out_unit(b, 0, mo))
                            if j == 4 and b + 1 < NB:
                                push(("q", b + 1, 0, 0), q_unit(b + 1, 0, 0))
                                push(("k", b + 1, 0, 0), k_unit(b + 1, 0, 0))
                                push(("k", b + 1, 0, 1), k_unit(b + 1, 0, 1))
                        if b == 1 and c == 0 and j == 0:
                            for mo in range(KD):
                                push(("o", 0, 1, mo), out_unit(0, 1, mo))
                        attention_cj(b, c, j)

            # ---- tail: last batch / last chunk output projection ----
            drain(len(fill))
            for mo in range(KD):
                out_unit(NB - 1, NCH - 1, mo)()

    nc.compile()
    return nc


def _get_nc():
    global _NC_CACHE
    if _NC_CACHE is None:
        _NC_CACHE = build_nc()
    return _NC_CACHE


def kernel(hidden_states, Wq, bq, Wk, bk, Wv, bv, Wo, bo):
    global LAST_RESULTS
    bf = ml_dtypes.bfloat16
    hs = np.asarray(hidden_states, np.float32)
    Wq = np.asarray(Wq, np.float32)
    Wk = np.asarray(Wk, np.float32)
    Wv = np.asarray(Wv, np.float32)
    Wo = np.asarray(Wo, np.float32)
    bq = np.asarray(bq, np.float32)
    bk = np.asarray(bk, np.float32)
    bv = np.asarray(bv, np.float32)
    bo = np.asarray(bo, np.float32)

    wqt = np.ascontiguousarray(Wq.T).astype(bf)
    wkt = np.ascontiguousarray(Wk.T).astype(bf)
    wvt = np.ascontiguousarray(Wv.T).astype(bf)
    wot = np.ascontiguousarray(Wo.T).astype(bf)
    bo2 = (bo + Wo @ bv).astype(np.float32)

    bpc = hs.shape[0] // NCORES  # batches per core
    in_maps = []
    for c in range(NCORES):
        xc = hs[c * bpc : (c + 1) * bpc].reshape(bpc * TB, D)
        in_maps.append(
            {
                "xt": np.ascontiguousarray(xc.T).astype(bf),
                "wqt": wqt,
                "wkt": wkt,
                "wvt": wvt,
                "wot": wot,
                "bq": bq,
                "bk": bk,
                "bo2": bo2,
            }
        )

    nc = _get_nc()
    res = run_bass_kernel_spmd(
        nc,
        in_maps,
        core_ids=list(range(NCORES)),
        trace=TRACE,
        **TRACE_KWARGS,
    )
    LAST_RESULTS = res

    out = np.empty((hs.shape[0], TB, D), np.float32)
    for c in range(NCORES):
        ot = res.results[c]["outt"]  # [D, T]
        for b in range(bpc):
            out[c * bpc + b] = ot[:, b * TB : (b + 1) * TB].T
    return out


# revision 20
# speedup vs baseline: 1.0964x; 1.0279x over previous
"""Multi-head attention forward (B=16, S=1024, d=1024, H=16, Dh=64) on 8
Trainium2 NeuronCores, data-parallel over batch (2 batches per core).

Device kernel (per core, bf16 matmuls, fp32 accumulate):
  inputs (host-prepped): XT [d, 2048] = hidden[2c:2c+2].reshape(2048,d).T,
  WqT/WkT/WvT = W.T [in, out], WoT = Wo.T [dv, o]  (all bf16),
  bq, bk [1024] f32, bo2 = bo + Wo @ bv  (bv folded: softmax rows sum to 1).

Key structure (vs earlier version): query chunks (c) are OUTER, head pairs
(j) inner.  Scores for a head pair go into ONE [128,1024] PSUM tile
(h0 -> bank A cols 0:512, h1 -> bank B) so the two 64x128 row-tiled matmuls
become ready together and execute CONCURRENTLY on the PE (row tiles 0/64),
and a single [128,1024] exp drains both.  pvps needs only 2 banks per (c,j)
so PSUM = scores ring 4 + pv 2 + proj 2 = 8 banks.  Projections / V-proj /
out-proj are emitted as 8-matmul fill units drained between score blocks.
"""

import numpy as np
import ml_dtypes

import concourse.bass as bass
import concourse.mybir as mybir
import concourse.tile as tile
from concourse import bacc
from concourse.bass_utils import run_bass_kernel_spmd

P = 128
D = 1024
T = 2048  # tokens per core
TB = 1024  # tokens per batch (= S)
H = 16
DH = 64
KD = D // P  # 8 partition-tiles of the d/dv/s dims
NB = T // TB  # batches per core
CW = 512  # query-chunk width (one PSUM bank of fp32)
NCH = TB // CW  # 2 query chunks per batch
NCORES = 8

BF16 = mybir.dt.bfloat16
F32 = mybir.dt.float32
EXPF = mybir.ActivationFunctionType.Exp
MULT = mybir.AluOpType.mult

# test.py hooks
TRACE = False
TRACE_KWARGS = {}
LAST_RESULTS = None

_NC_CACHE = None


def build_nc():
    from collections import deque
    from contextlib import ExitStack

    nc = bacc.Bacc("TRN2", target_bir_lowering=False, debug=False, num_devices=NCORES)

    xt_d = nc.dram_tensor("xt", [D, T], BF16, kind="ExternalInput")
    wqt_d = nc.dram_tensor("wqt", [D, D], BF16, kind="ExternalInput")
    wkt_d = nc.dram_tensor("wkt", [D, D], BF16, kind="ExternalInput")
    wvt_d = nc.dram_tensor("wvt", [D, D], BF16, kind="ExternalInput")
    wot_d = nc.dram_tensor("wot", [D, D], BF16, kind="ExternalInput")
    bq_d = nc.dram_tensor("bq", [D], F32, kind="ExternalInput")
    bk_d = nc.dram_tensor("bk", [D], F32, kind="ExternalInput")
    bo2_d = nc.dram_tensor("bo2", [D], F32, kind="ExternalInput")
    outt_d = nc.dram_tensor("outt", [D, T], F32, kind="ExternalOutput")

    with tile.TileContext(nc) as tc:
        with ExitStack() as ctx:
            # PSUM first so the 2-bank scores tiles land bank-aligned.
            scp = ctx.enter_context(tc.tile_pool(name="sc", bufs=2, space="PSUM"))
            pvp = ctx.enter_context(tc.tile_pool(name="pv", bufs=1, space="PSUM"))
            prp = ctx.enter_context(tc.tile_pool(name="pr", bufs=2, space="PSUM"))
            wpool = ctx.enter_context(tc.tile_pool(name="w", bufs=1))
            xpool = ctx.enter_context(tc.tile_pool(name="x", bufs=1))
            qkpool = ctx.enter_context(tc.tile_pool(name="qk", bufs=1))
            vpool = ctx.enter_context(tc.tile_pool(name="v", bufs=2))
            ptpool = ctx.enter_context(tc.tile_pool(name="pt", bufs=4))
            cpool = ctx.enter_context(tc.tile_pool(name="ctx", bufs=2))
            npool = ctx.enter_context(tc.tile_pool(name="norm", bufs=2))
            opool = ctx.enter_context(tc.tile_pool(name="out", bufs=2))
            spool = ctx.enter_context(tc.tile_pool(name="small", bufs=1))

            # ---- global tiles + DMA loads (interleaved for fast rampup) ----
            xt = [xpool.tile([P, T], BF16, tag=f"xt{k}", name=f"xt{k}") for k in range(KD)]
            wq, wk, wv, wo = (
                [wpool.tile([P, D], BF16, tag=f"w{nm}{k}", name=f"w{nm}{k}") for k in range(KD)]
                for nm in "qkvo"
            )
            # batch-0 halves of xt + wq/wk first (unblocks first QK proj),
            # then wv (V-proj), then batch-1 xt halves, wo last.
            for k in range(KD):
                nc.sync.dma_start(xt[k][:, 0:TB], xt_d[k * P : (k + 1) * P, 0:TB])
                nc.sync.dma_start(wq[k][:], wqt_d[k * P : (k + 1) * P, :])
                nc.sync.dma_start(wk[k][:], wkt_d[k * P : (k + 1) * P, :])
            bq_sb = spool.tile([P, KD], F32, tag="bq", name="bq_sb")
            bk_sb = spool.tile([P, KD], F32, tag="bk", name="bk_sb")
            bo_sb = spool.tile([P, KD], F32, tag="bo", name="bo_sb")
            for sb, dr in ((bq_sb, bq_d), (bk_sb, bk_d)):
                nc.sync.dma_start(sb[:], dr.rearrange("(o p) -> p o", p=P))
            for k in range(KD):
                nc.sync.dma_start(wv[k][:], wvt_d[k * P : (k + 1) * P, :])
            nc.sync.dma_start(bo_sb[:], bo2_d.rearrange("(o p) -> p o", p=P))
            for k in range(KD):
                nc.sync.dma_start(xt[k][:, TB:T], xt_d[k * P : (k + 1) * P, TB:T])
            for k in range(KD):
                nc.sync.dma_start(wo[k][:], wot_d[k * P : (k + 1) * P, :])

            # ---- persistent per-batch state ----
            vt = {}  # (b, mt) -> v tile [P, H, DH+1]
            kt = {}  # (b, j) -> K^T tile [P, TB]
            qt = {}  # (b, j, c) -> Q^T chunk tile [P, CW]
            ctxts = {}  # (b, j) -> ctx^T tile [P, TB]

            fill = deque()  # (key, closure)
            done = set()

            def push(key, closure):
                fill.append((key, closure))

            def drain(n):
                for _ in range(min(n, len(fill))):
                    k, f = fill.popleft()
                    f()
                    done.add(k)

            def need(key):
                # force-drain (in FIFO order) until `key` has been emitted;
                # guarantees emission-order dependencies for dict tiles.
                while key not in done:
                    assert fill, f"need({key}) but fill queue empty"
                    k, f = fill.popleft()
                    f()
                    done.add(k)

            # ---- fill units (each ~8 matmuls + epilogue) ----
            def v_unit(b, mt, ch):
                def emit():
                    if (b, mt) not in vt:
                        vt[(b, mt)] = vpool.tile(
                            [P, H, DH + 1], BF16, tag=f"v{mt}", name=f"v{mt}", bufs=2
                        )
                        nc.vector.memset(vt[(b, mt)][:, :, DH : DH + 1], 1.0)
                    ps = prp.tile([P, CW], F32, tag="pr", name="prv")
                    for k in range(KD):
                        nc.tensor.matmul(
                            ps[:],
                            xt[k][:, (b * KD + mt) * P : (b * KD + mt + 1) * P],
                            wv[k][:, ch * CW : (ch + 1) * CW],
                            start=(k == 0),
                            stop=(k == KD - 1),
                        )
                    nc.vector.tensor_copy(
                        vt[(b, mt)][:, ch * 8 : (ch + 1) * 8, 0:DH],
                        ps.rearrange("p (h d) -> p h d", d=DH),
                    )

                return emit

            def k_unit(b, j, ch):
                def emit():
                    if (b, j) not in kt:
                        kt[(b, j)] = qkpool.tile(
                            [P, TB], BF16, tag=f"k{j}", name=f"kt{j}", bufs=1
                        )
                    ps = prp.tile([P, CW], F32, tag="pr", name="prk")
                    for k in range(KD):
                        nc.tensor.matmul(
                            ps[:],
                            wk[k][:, j * P : (j + 1) * P],
                            xt[k][:, b * TB + ch * CW : b * TB + (ch + 1) * CW],
                            start=(k == 0),
                            stop=(k == KD - 1),
                        )
                    nc.vector.tensor_scalar_add(
                        kt[(b, j)][:, ch * CW : (ch + 1) * CW], ps[:], bk_sb[:, j : j + 1]
                    )

                return emit

            def q_unit(b, j, c):
                def emit():
                    qt[(b, j, c)] = qkpool.tile(
                        [P, CW], BF16, tag=f"q{j}", name=f"qt{j}", bufs=1
                    )
                    ps = prp.tile([P, CW], F32, tag="pr", name="prq")
                    for k in range(KD):
                        nc.tensor.matmul(
                            ps[:],
                            wq[k][:, j * P : (j + 1) * P],
                            xt[k][:, b * TB + c * CW : b * TB + (c + 1) * CW],
                            start=(k == 0),
                            stop=(k == KD - 1),
                        )
                    nc.vector.tensor_scalar_add(
                        qt[(b, j, c)][:], ps[:], bq_sb[:, j : j + 1]
                    )

                return emit

            def out_unit(b, c, mo):
                def emit():
                    ps = prp.tile([P, CW], F32, tag="pr", name="pro")
                    for k in range(KD):
                        nc.tensor.matmul(
                            ps[:],
                            wo[k][:, mo * P : (mo + 1) * P],
                            ctxts[(b, k)][:, c * CW : (c + 1) * CW],
                            start=(k == 0),
                            stop=(k == KD - 1),
                        )
                    osb = opool.tile([P, CW], F32, tag="osb", name="osb")
                    nc.vector.tensor_scalar_add(osb[:], ps[:], bo_sb[:, mo : mo + 1])
                    nc.sync.dma_start(
                        outt_d[
                            mo * P : (mo + 1) * P,
                            b * TB + c * CW : b * TB + (c + 1) * CW,
                        ],
                        osb[:],
                    )

                return emit

            # ---- attention inner loop ----
            def normalize(b, c, j, pva, pvb):
                if (b, j) not in ctxts:
                    ctxts[(b, j)] = cpool.tile(
                        [P, TB], BF16, tag=f"ctxt{j}", name=f"ctxt{j}", bufs=2
                    )
                ctile = ctxts[(b, j)]
                for h, pv_t in ((0, pva), (1, pvb)):
                    rs = npool.tile([1, CW], F32, tag="rs", name="rs", bufs=1)
                    nc.vector.tensor_copy(rs[:], pv_t[DH : DH + 1, :])
                    rr = npool.tile([1, CW], F32, tag="rr", name="rr", bufs=1)
                    nc.vector.reciprocal_approx_fast(rr[:], rs[:])
                    rb = npool.tile([DH, CW], F32, tag="rb", name="rb", bufs=2)
                    nc.gpsimd.partition_broadcast(rb[:], rr[:])
                    if h == 0:
                        nc.vector.tensor_tensor(
                            ctile[0:DH, c * CW : (c + 1) * CW],
                            pv_t[0:DH, :],
                            rb[:],
                            MULT,
                        )
                    else:
                        ch = npool.tile([DH, CW], BF16, tag="ch", name="ch", bufs=2)
                        nc.vector.tensor_tensor(ch[:], pv_t[0:DH, :], rb[:], MULT)
                        nc.sync.dma_start(
                            ctile[DH:P, c * CW : (c + 1) * CW], ch[:]
                        )

            def attention_cj(b, c, j):
                need(("q", b, j, c))
                need(("k", b, j, 0))
                need(("k", b, j, 1))
                pva = pvp.tile([P, CW], F32, tag="pva", name="pva", bufs=1)
                pvb = pvp.tile([P, CW], F32, tag="pvb", name="pvb", bufs=1)
                ktj = kt[(b, j)]
                qjc = qt[(b, j, c)]
                pts = []
                for blk in range(KD // 2):
                    two = (2 * blk, 2 * blk + 1)
                    scs = []
                    for st in two:
                        sc = scp.tile([P, 2 * CW], F32, tag="sc", name="sc", bufs=2)
                        for h in range(2):
                            r0 = h * DH
                            nc.tensor.matmul(
                                sc[:, h * CW : (h + 1) * CW],
                                ktj[r0 : r0 + DH, st * P : (st + 1) * P],
                                qjc[r0 : r0 + DH, :],
                                start=True,
                                stop=True,
                            )
                        scs.append(sc)
                    for sc in scs:
                        pt = ptpool.tile([P, 2 * CW], BF16, tag="pt", name="pt", bufs=3)
                        nc.scalar.activation(pt[:], sc[:], EXPF, scale=0.125)
                        pts.append(pt)
                    drain(1)
                    for st in two:
                        need(("v", b, st, j // 4))
                        for h, pv_t in ((0, pva), (1, pvb)):
                            nc.tensor.matmul(
                                pv_t[0 : DH + 1, :],
                                vt[(b, st)][:, 2 * j + h, :],
                                pts[st][:, h * CW : (h + 1) * CW],
                                start=(st == 0),
                                stop=(st == KD - 1),
                            )
                normalize(b, c, j, pva, pvb)

            # ---- head: QK proj of (b0, j0) emitted directly; V + j1 proj
            # queued so early score pairs preempt them by priority ----
            for key, u in (
                (("q", 0, 0, 0), q_unit(0, 0, 0)),
                (("k", 0, 0, 0), k_unit(0, 0, 0)),
                (("k", 0, 0, 1), k_unit(0, 0, 1)),
            ):
                u()
                done.add(key)
            push(("v", 0, 0, 0), v_unit(0, 0, 0))
            push(("v", 0, 1, 0), v_unit(0, 1, 0))
            push(("q", 0, 1, 0), q_unit(0, 1, 0))
            push(("k", 0, 1, 0), k_unit(0, 1, 0))
            push(("k", 0, 1, 1), k_unit(0, 1, 1))
            for mt in range(2, KD):
                push(("v", 0, mt, 0), v_unit(0, mt, 0))
            for mt in range(KD):
                push(("v", 0, mt, 1), v_unit(0, mt, 1))

            # ---- main loops ----
            for b in range(NB):
                for c in range(NCH):
                    for j in range(KD):
                        # schedule fill production
                        if c == 0:
                            if j < KD - 1:
                                if not (b == 0 and j == 0):  # j1 pre-queued in head
                                    push(("q", b, j + 1, 0), q_unit(b, j + 1, 0))
                                    push(("k", b, j + 1, 0), k_unit(b, j + 1, 0))
                                    push(("k", b, j + 1, 1), k_unit(b, j + 1, 1))
                            else:
                                push(("q", b, 0, 1), q_unit(b, 0, 1))
                        else:
                            if j < KD - 1:
                                push(("q", b, j + 1, 1), q_unit(b, j + 1, 1))
                            if j == 0 and b + 1 < NB:
                                # next batch V-proj + this batch's c0 out-proj
                                for mt in range(KD):
                                    push(("v", b + 1, mt, 0), v_unit(b + 1, mt, 0))
                                    if mt % 2 == 0:
                                        push(("o", b, 0, mt // 2), out_unit(b, 0, mt // 2))
                                for mt in range(KD):
                                    push(("v", b + 1, mt, 1), v_unit(b + 1, mt, 1))
                                    if mt % 2 == 1:
                                        push(("o", b, 0, mt // 2 + 4), out_unit(b, 0, mt // 2 + 4))
                            if j == 4 and b + 1 == NB:
                                # late so the final phase has PE fill while the
                                # last normalizes drain
                                for mo in range(KD):
                                    push(("o", b, 0, mo), out_unit(b, 0, mo))
                            if j == 4 and b + 1 < NB:
                                push(("q", b + 1, 0, 0), q_unit(b + 1, 0, 0))
                                push(("k", b + 1, 0, 0), k_unit(b + 1, 0, 0))
                                push(("k", b + 1, 0, 1), k_unit(b + 1, 0, 1))
                        if b == 1 and c == 0 and j == 0:
                            for mo in range(KD):
                                push(("o", 0, 1, mo), out_unit(0, 1, mo))
                        attention_cj(b, c, j)

            # ---- tail: last batch / last chunk output projection ----
            drain(len(fill))
            for mo in range(KD):
                out_unit(NB - 1, NCH - 1, mo)()

    nc.compile()
    return nc


def _get_nc():
    global _NC_CACHE
    if _NC_CACHE is None:
        _NC_CACHE = build_nc()
    return _NC_CACHE


def kernel(hidden_states, Wq, bq, Wk, bk, Wv, bv, Wo, bo):
    global LAST_RESULTS
    bf = ml_dtypes.bfloat16
    hs = np.asarray(hidden_states, np.float32)
    Wq = np.asarray(Wq, np.float32)
    Wk = np.asarray(Wk, np.float32)
    Wv = np.asarray(Wv, np.float32)
    Wo = np.asarray(Wo, np.float32)
    bq = np.asarray(bq, np.float32)
    bk = np.asarray(bk, np.float32)
    bv = np.asarray(bv, np.float32)
    bo = np.asarray(bo, np.float32)

    wqt = np.ascontiguousarray(Wq.T).astype(bf)
    wkt = np.ascontiguousarray(Wk.T).astype(bf)
    wvt = np.ascontiguousarray(Wv.T).astype(bf)
    wot = np.ascontiguousarray(Wo.T).astype(bf)
    bo2 = (bo + Wo @ bv).astype(np.float32)

    bpc = hs.shape[0] // NCORES  # batches per core
    in_maps = []
    for c in range(NCORES):
        xc = hs[c * bpc : (c + 1) * bpc].reshape(bpc * TB, D)
        in_maps.append(
            {
                "xt": np.ascontiguousarray(xc.T).astype(bf),
                "wqt": wqt,
                "wkt": wkt,
                "wvt": wvt,
                "wot": wot,
                "bq": bq,
                "bk": bk,
                "bo2": bo2,
            }
        )

    nc = _get_nc()
    res = run_bass_kernel_spmd(
        nc,
        in_maps,
        core_ids=list(range(NCORES)),
        trace=TRACE,
        **TRACE_KWARGS,
    )
    LAST_RESULTS = res

    out = np.empty((hs.shape[0], TB, D), np.float32)
    for c in range(NCORES):
        ot = res.results[c]["outt"]  # [D, T]
        for b in range(bpc):
            out[c * bpc + b] = ot[:, b * TB : (b + 1) * TB].T
    return out
